# revision 5
# baseline (speedup 1.0000x reference)
"""Trainium2 Bass kernel for nn_Block_79680233275670 (dense transformer block).

Reference, for x [16, 1024, 384]:
  x = x + proj(attn(LN1(x)))                               (4 heads, head_dim 96)
  x = x + fc2(hswish(dw3x3(hswish(fc1(LN2(x))))))          (IRB, 32x32 spatial)

Sharding: pure data-parallel over batch B=16 -> 8 cores x 2 batch items.
No collectives. Weights replicated (pre-transposed / LN-folded / fp8-quantized
host-side).

Per-core dataflow (T = 2048 tokens = 2 batches x 1024):
  - x token-major [128, 16, 384] f32 (4 chunked DMAs); residual stream f32
  - LN token-major (bn_stats) -> bf16 -> DMA-XBAR transpose -> Pool fp8 cast
    (channel-major activations feed fp8 matmuls)
  - fp8e4 DoubleRow matmuls (2 k-tiles/pass) for QKV / PV / proj / fc1 / fc2;
    scores q^T k stay bf16
  - St pairs: one [128, 2048] PSUM holds scores of two 128-token m-tiles;
    one big exp on ACT emits the fp8 P pair = the DoubleRow rhs for PV.
    An appended ones column in v (padded to 112 rows for XBAR/DR alignment)
    makes PV also emit softmax denominators
  - deferred O normalization: DMA-transpose [112,1024] -> one DVE recip +
    per-slice Pool multiplies
  - proj/fc2 token-major; fp8 dequant scale folded into the
    scalar_tensor_tensor residual add (x += psum*s)
  - IRB: fc1 channel-major fp8; hardswish = ONE custom DVE op
    (min(relu(x*s+b+3),6)*(relu(..)-3), /6 folded into dw / fc2 weights)
    writing fp8 windows directly; depthwise 3x3 on PE as 9 diagonal-matmul
    taps over 19-row zero-padded windows (uniform tap geometry, no edge
    cases)
"""

import sys
import functools

for _p in ("/opt/trn_rl_repo",):
    if _p not in sys.path:
        sys.path.insert(0, _p)

import numpy as np
import ml_dtypes

import concourse.bass as bass
import concourse.mybir as mybir
import concourse.tile as tile
from concourse import bacc
from concourse.masks import make_identity

B, N, C = 16, 1024, 384
HEADS, HD = 4, 96
VP = 112                   # v rows incl. ones col, padded for XBAR/DR align
HID = 1536
NCORES = 8
BPC = B // NCORES          # batches per core
T = BPC * N                # tokens per core
NT = T // 128              # 16 token tiles per core
EPS = 1e-5

f32 = mybir.dt.float32
bf16 = mybir.dt.bfloat16
fp8 = mybir.dt.float8e4
AF = mybir.ActivationFunctionType
OP = mybir.AluOpType
DR = mybir.MatmulPerfMode.DoubleRow
nbf = ml_dtypes.bfloat16
nf8 = ml_dtypes.float8_e4m3

# ---- custom fused hardswish DVE op (registered at import time) ------------
# out = min(relu(in*C1 + C0), 6) * (relu(in*C1 + C0) - 3)
#     = hswish6(in*C1 + (C0-3)) where hswish6(x) = x*clip(x+3,0,6)
# C0 = bias+3 (per-partition AP), C1 = fp8 dequant scale, C2 = 6 (imm2),
# C3 = 3 delivered via in1 (spilled).
import concourse.dve_ops as dve_ops
from concourse.dve_spec import Spec, Src0, C0, C1, C2, relu, minn, lower
from concourse.dve_ops import DveOp, DveOpSpec, _spill_c3_to_src1


def _register_hswish():
    name = "HSWISH6Q_ANT"
    if name in dve_ops._SUB_OPCODE_FOR_NAME:
        for op in dve_ops.OPS:
            if op.name == name:
                return op
    r = relu(Src0 * C1 + C0)
    spec = Spec(
        body=_spill_c3_to_src1(minn(r, C2) * (r - C3)),
        reference=lambda in0, in1, s0, s1, imm2:
        np.minimum(np.maximum(in0 * s1 + s0, 0), imm2)
        * (np.maximum(in0 * s1 + s0, 0) - in1),
    )
    op = DveOp(name, spec, subdim=False, uops_sha={})
    row = dve_ops._CUSTOM_DVE_ROW_BASE + len(dve_ops.OPS)
    assert row < 0x20
    for ver in ("v3", "v4"):
        probe = DveOpSpec(name=name, opcode=row, uops=lower(spec, ver=ver),
                          rd1_en=True)
        op.uops_sha[ver] = probe.sha(ver)
    dve_ops.OPS.append(op)
    dve_ops._SUB_OPCODE_FOR_NAME[name] = row
    dve_ops.CUSTOM_DVE_SPECS[name] = spec
    return op


from concourse.dve_spec import C3  # noqa: E402  (after Spec imports)

HSWISH6Q = _register_hswish()

# ---- depthwise-window geometry -------------------------------------------
# 19 rows per window: row 0 and row 18 stay zero (vertical SAME padding),
# rows 1..17 hold 17 image rows (16 outputs + 1 halo). Every row is padded
# to WP=34 (32 data + 2 zero cols -> horizontal SAME padding), plus HOFF=2
# leading zeros. With the zero rows, all 9 taps share identical geometry:
#   acc[0:542) += w_t * win[so(t) : so(t)+542),  so(t) = HOFF+(dy+1)*WP+dx
WROWS = 17          # data rows per window
WP = 34
HOFF = 2
HLEN = HOFF + 19 * WP      # 648
WTOK = WROWS * 32          # 544 tokens of fc1 output per window
ACCL = 16 * WP             # 544 acc length
AUSE = ACCL - 2            # 542 initialized prefix
SEGS = ((0, 512), (512, AUSE))   # PSUM-bank-safe column segments


def tap_off(t):
    dy, dx = t // 3 - 1, t % 3 - 1
    return HOFF + (dy + 1) * WP + dx


def emit_kernel(nc, tc, d):
    from contextlib import ExitStack

    with ExitStack() as ctx:
        singles = ctx.enter_context(tc.tile_pool(name="singles", bufs=1))

        x_sb = singles.tile([128, NT, C], f32)   # token-major; becomes x2 in place
        ones8 = singles.tile([1, 128], fp8)
        nc.vector.memset(ones8, 1.0)
        eps_sb = singles.tile([128, 1], f32)
        nc.vector.memset(eps_sb, EPS)
        three = singles.tile([128, 1], f32)
        nc.vector.memset(three, 3.0)

        wqk_sb = singles.tile([128, 3, 2 * C], fp8)
        bqk_sb = singles.tile([96, 8], f32)
        wv_sb = singles.tile([128, 3, C], fp8)
        bv_sb = singles.tile([1, C], fp8)
        wp_sb = singles.tile([128, 3, C], fp8)
        bp_sb = singles.tile([1, C], fp8)
        wf1_sb = singles.tile([128, 3, HID], fp8)
        bf13_sb = singles.tile([128, 12], f32)
        wdg_sb = singles.tile([128, 12, 9, 128], fp8)
        bdw3_sb = singles.tile([128, 12], f32)
        wf2_sb = singles.tile([128, 12, C], fp8)
        bf2_sb = singles.tile([1, C], fp8)

        h1w_a = singles.tile([128, 12, HLEN], fp8)
        h1w_b = singles.tile([128, 12, HLEN], fp8)
        h1w_bufs = [h1w_a, h1w_b]
        nc.gpsimd.memset(h1w_a, 0.0)
        nc.gpsimd.memset(h1w_b, 0.0)

        xn_chT = singles.tile([128, 3, T], bf16)  # shared LN1/LN2 staging

        for name, dst in (("wqk", wqk_sb), ("wv", wv_sb), ("wp", wp_sb),
                          ("wf1", wf1_sb)):
            nc.sync.dma_start(out=dst, in_=d[name].rearrange("k p m -> p k m"))
        nc.sync.dma_start(out=wf2_sb, in_=d["wf2"].rearrange("k p m -> p k m"))
        nc.sync.dma_start(
            out=wdg_sb, in_=d["wdg"].rearrange("m t c j -> c m t j"))
        for name, dst in (("bqk", bqk_sb), ("bv", bv_sb), ("bp", bp_sb),
                          ("bf13", bf13_sb), ("bdw3", bdw3_sb),
                          ("bf2", bf2_sb)):
            nc.sync.dma_start(out=dst, in_=d[name])

        # x[b, i*128+p, c] -> x_sb[p, b*8+i, c], 4 chunks so LN1 starts early
        for ch in range(4):
            nc.sync.dma_start(
                out=x_sb[:, ch * 4:(ch + 1) * 4, :],
                in_=d["x"].rearrange("b (i p) c -> p (b i) c", p=128)
                [:, ch * 4:(ch + 1) * 4, :],
            )

        SQ = d["scales"]  # dict of python floats

        def layernorm_to_ch(xn_ch, ln_pool):
            for tt in range(NT):
                stats = ln_pool.tile([128, 6], f32, tag="ln_stats")
                nc.vector.bn_stats(stats, x_sb[:, tt, :])
                mv = ln_pool.tile([128, 2], f32, tag="ln_mv")
                nc.vector.bn_aggr(mv, stats)
                std = ln_pool.tile([128, 1], f32, tag="ln_std")
                nc.scalar.activation(std, mv[:, 1:2], AF.Sqrt, bias=eps_sb)
                rstd = ln_pool.tile([128, 1], f32, tag="ln_rstd")
                nc.vector.reciprocal(rstd, std)
                xn = ln_pool.tile([128, C], bf16, tag="ln_xn")
                nc.vector.tensor_scalar(
                    xn, x_sb[:, tt, :], mv[:, 0:1], rstd, OP.subtract, OP.mult
                )
                tsl = slice(tt * 128, (tt + 1) * 128)
                nc.sync.dma_start_transpose(xn_chT[:, :, tsl], xn)
                nc.gpsimd.tensor_copy(xn_ch[:, :, tsl], xn_chT[:, :, tsl])

        # ======================= attention =======================
        with tc.tile_pool(name="attn_acts", bufs=1) as apool:
            xn1_ch = apool.tile([128, 3, T], fp8)
            q_sb = apool.tile([96, HEADS, T], bf16)
            k_sb = apool.tile([96, HEADS, T], bf16)
            v_sb = apool.tile([128, NT, HEADS, VP], fp8)
            o_norm = apool.tile([128, NT, HEADS, HD], bf16)
            o_ch = apool.tile([128, 3, T], fp8)
            nc.vector.memset(v_sb[:, :, :, HD:HD + 1], 1.0)
            nc.vector.memset(v_sb[:, :, :, HD + 1:VP], 0.0)

            with tc.tile_pool(name="ln1", bufs=3) as ln_pool:
                layernorm_to_ch(xn1_ch, ln_pool)

            with tc.tile_pool(name="qk_ps", bufs=2, space="PSUM") as qk_ps, \
                 tc.tile_pool(name="v_ps", bufs=2, space="PSUM") as v_ps:
                for io in range(2):  # 0=q, 1=k
                    dst = q_sb if io == 0 else k_sb
                    for h in range(HEADS):
                        co = io * C + h * HD
                        for cn in range(T // 1024):
                            ps = qk_ps.tile([96, 1024], f32, tag="qk")
                            for half in range(2):
                                tsl = slice(cn * 1024 + half * 512,
                                            cn * 1024 + half * 512 + 512)
                                psl = slice(half * 512, half * 512 + 512)
                                nc.tensor.matmul(
                                    ps[:, psl], wqk_sb[:, 0:2, co:co + HD],
                                    xn1_ch[:, 0:2, tsl],
                                    start=True, stop=False, perf_mode=DR,
                                )
                                nc.tensor.matmul(
                                    ps[:, psl], wqk_sb[:, 2, co:co + HD],
                                    xn1_ch[:, 2, tsl], start=False, stop=True,
                                )
                            nc.scalar.activation(
                                dst[:, h, cn * 1024:(cn + 1) * 1024], ps,
                                AF.Identity,
                                bias=bqk_sb[:, io * 4 + h: io * 4 + h + 1],
                                scale=1.0 / SQ["qk"],
                            )
                for tt in range(NT):
                    ps = v_ps.tile([128, C], f32, tag="v")
                    tsl = slice(tt * 128, (tt + 1) * 128)
                    nc.tensor.matmul(
                        ps, xn1_ch[:, 0:2, tsl], wv_sb[:, 0:2, :],
                        start=True, stop=False, perf_mode=DR,
                    )
                    nc.tensor.matmul(ps, xn1_ch[:, 2, tsl], wv_sb[:, 2, :],
                                     start=False, stop=False)
                    nc.tensor.matmul(ps, ones8, bv_sb, start=False, stop=True)
                    nc.scalar.activation(
                        v_sb[:, tt, :, 0:HD],
                        ps.rearrange("p (h e) -> p h e", h=HEADS),
                        AF.Copy, scale=1.0 / SQ["v"],
                    )

            ou_tiles = {}
            with tc.tile_pool(name="st_ps", bufs=1, space="PSUM") as st_ps, \
                 tc.tile_pool(name="o_ps", bufs=2, space="PSUM") as o_ps, \
                 tc.tile_pool(name="pt_pool", bufs=2) as pt_pool, \
                 tc.tile_pool(name="ou_pool", bufs=1) as ou_pool:
                for b in range(BPC):
                    for h in range(HEADS):
                        o_psum = o_ps.tile([VP, N], f32, tag="o")
                        for mp in range(4):
                            st = st_ps.tile([128, 2 * N], f32, tag="st")
                            for i in range(2):
                                mt = 2 * mp + i
                                for cn in range(2):
                                    nc.tensor.matmul(
                                        st[:, i * N + cn * 512:
                                           i * N + (cn + 1) * 512],
                                        k_sb[:, h, b * N + mt * 128:
                                             b * N + (mt + 1) * 128],
                                        q_sb[:, h, b * N + cn * 512:
                                             b * N + (cn + 1) * 512],
                                        start=True, stop=True,
                                    )
                            pt2 = pt_pool.tile([128, 2, N], fp8, tag="pt")
                            nc.scalar.activation(
                                pt2.rearrange("p i n -> p (i n)"), st, AF.Exp)
                            vp = v_sb[:, b * 8 + 2 * mp: b * 8 + 2 * mp + 2,
                                      h, :]
                            for cn in range(2):
                                nc.tensor.matmul(
                                    o_psum[:, cn * 512:(cn + 1) * 512],
                                    vp, pt2[:, :, cn * 512:(cn + 1) * 512],
                                    start=(mp == 0), stop=(mp == 3),
                                    perf_mode=DR, skip_group_check=True,
                                )
                        o_un = ou_pool.tile([VP, N], bf16, tag=f"ou{b}{h}")
                        nc.scalar.activation(o_un, o_psum, AF.Copy)
                        ou_tiles[(b, h)] = o_un

            with tc.tile_pool(name="tpo_pool", bufs=2) as tpo_pool, \
                 tc.tile_pool(name="r_pool", bufs=4) as r_pool, \
                 tc.tile_pool(name="ot_pool", bufs=3) as ot_pool, \
                 tc.tile_pool(name="pj_ps", bufs=3, space="PSUM") as pj_ps:
                # deferred O normalization (DMA transpose + one recip)
                for b in range(BPC):
                    for h in range(HEADS):
                        tpo = tpo_pool.tile([128, 8, VP], bf16, tag="tpo")
                        nc.sync.dma_start_transpose(tpo, ou_tiles[(b, h)])
                        r8 = r_pool.tile([128, 8], f32, tag="r")
                        nc.vector.reciprocal(r8, tpo[:, :, HD])
                        for ns in range(8):
                            nc.gpsimd.tensor_scalar(
                                o_norm[:, b * 8 + ns, h, :], tpo[:, ns, 0:HD],
                                r8[:, ns:ns + 1], None, OP.mult,
                            )
                for tt in range(NT):
                    ot = ot_pool.tile([128, 3, 128], bf16, tag="ot")
                    ov = o_norm[:, tt, :, :].rearrange("p h e -> p (h e)")
                    nc.sync.dma_start_transpose(ot, ov)
                    nc.gpsimd.tensor_copy(
                        o_ch[:, :, tt * 128:(tt + 1) * 128], ot)
                for tt in range(NT):
                    ps = pj_ps.tile([128, C], f32, tag="pj")
                    tsl = slice(tt * 128, (tt + 1) * 128)
                    nc.tensor.matmul(
                        ps, o_ch[:, 0:2, tsl], wp_sb[:, 0:2, :],
                        start=True, stop=False, perf_mode=DR,
                    )
                    nc.tensor.matmul(ps, o_ch[:, 2, tsl], wp_sb[:, 2, :],
                                     start=False, stop=False)
                    nc.tensor.matmul(ps, ones8, bp_sb, start=False, stop=True)
                    nc.vector.scalar_tensor_tensor(
                        x_sb[:, tt, :], ps, 1.0 / SQ["p"], x_sb[:, tt, :],
                        OP.mult, OP.add,
                    )

        # ======================= IRB branch =======================
        with tc.tile_pool(name="irb_acts", bufs=1) as npool:
            xn2_ch = npool.tile([128, 3, T], fp8)

            with tc.tile_pool(name="ln2", bufs=3) as ln_pool:
                layernorm_to_ch(xn2_ch, ln_pool)

            with tc.tile_pool(name="h2_pool", bufs=2) as h2_pool, \
                 tc.tile_pool(name="out_pool", bufs=4) as out_pool, \
                 tc.tile_pool(name="f1_ps", bufs=2, space="PSUM") as f1_ps, \
                 tc.tile_pool(name="dw_ps", bufs=1, space="PSUM") as dw_ps, \
                 tc.tile_pool(name="f2_ps", bufs=2, space="PSUM") as f2_ps:
                for b in range(BPC):
                    for yh in range(2):
                        wy0 = 0 if yh == 0 else 15  # first image row in window
                        tok0 = b * N + wy0 * 32
                        h1w = h1w_bufs[(b * 2 + yh) % 2]
                        for m in range(12):
                            ps = f1_ps.tile([128, WTOK], f32, tag="f1")
                            msl = slice(m * 128, (m + 1) * 128)
                            for c0, cw in ((0, 512), (512, WTOK - 512)):
                                csl = slice(tok0 + c0, tok0 + c0 + cw)
                                nc.tensor.matmul(
                                    ps[:, c0:c0 + cw],
                                    wf1_sb[:, 0:2, msl], xn2_ch[:, 0:2, csl],
                                    start=True, stop=False, perf_mode=DR,
                                )
                                nc.tensor.matmul(
                                    ps[:, c0:c0 + cw],
                                    wf1_sb[:, 2, msl], xn2_ch[:, 2, csl],
                                    start=False, stop=True,
                                )
                            # fused hardswish (x6; /6 folded into wdw), fp8 out
                            h1v = h1w[:, m, HOFF + WP:HOFF + WP + WROWS * WP] \
                                .rearrange("p (y x) -> p y x", x=WP)[:, :, 0:32]
                            nc.vector._custom_dve(
                                HSWISH6Q, out=h1v,
                                in0=ps.rearrange("p (y x) -> p y x", x=32),
                                in1=three, s0=bf13_sb[:, m:m + 1],
                                s1=1.0 / SQ["f1"], imm2=6.0,
                            )
                        for m in range(12):
                            dps = dw_ps.tile([128, ACCL], f32, tag="dwp")
                            for u0, u1 in SEGS:
                                for t in range(9):
                                    so = tap_off(t)
                                    nc.tensor.matmul(
                                        dps[:, u0:u1], wdg_sb[:, m, t, :],
                                        h1w[:, m, so + u0:so + u1],
                                        start=(t == 0), stop=(t == 8),
                                        skip_group_check=True,
                                    )
                            if m == 0:
                                h2 = h2_pool.tile([128, 12, 512], fp8, tag="h2")
                            nc.vector._custom_dve(
                                HSWISH6Q,
                                out=h2[:, m, :].rearrange(
                                    "p (y x) -> p y x", x=32),
                                in0=dps.rearrange(
                                    "p (y x) -> p y x", x=WP)[:, :, 0:32],
                                in1=three, s0=bdw3_sb[:, m:m + 1],
                                s1=1.0 / SQ["dw"], imm2=6.0,
                            )
                        # fc2 + residual (hswish2's /6 folded into wf2)
                        for tl in range(4):
                            tg = b * 8 + yh * 4 + tl
                            ps = f2_ps.tile([128, C], f32, tag="f2")
                            for mi in range(6):
                                nc.tensor.matmul(
                                    ps, h2[:, 2 * mi:2 * mi + 2,
                                           tl * 128:(tl + 1) * 128],
                                    wf2_sb[:, 2 * mi:2 * mi + 2, :],
                                    start=(mi == 0), stop=False, perf_mode=DR,
                                )
                            nc.tensor.matmul(ps, ones8, bf2_sb,
                                             start=False, stop=True)
                            ot = out_pool.tile([128, C], f32, tag="out")
                            nc.vector.scalar_tensor_tensor(
                                ot, ps, 1.0 / SQ["f2"], x_sb[:, tg, :],
                                OP.mult, OP.add,
                            )
                            nc.sync.dma_start(
                                out=d["out"][b,
                                             (yh * 4 + tl) * 128:
                                             (yh * 4 + tl + 1) * 128,
                                             :],
                                in_=ot,
                            )


def declare_tensors(nc, scales):
    d = {"scales": scales}
    d["x"] = nc.dram_tensor("x", [BPC, N, C], f32, kind="ExternalInput").ap()
    d["wqk"] = nc.dram_tensor("wqk", [3, 128, 2 * C], fp8, kind="ExternalInput").ap()
    d["bqk"] = nc.dram_tensor("bqk", [96, 8], f32, kind="ExternalInput").ap()
    d["wv"] = nc.dram_tensor("wv", [3, 128, C], fp8, kind="ExternalInput").ap()
    d["bv"] = nc.dram_tensor("bv", [1, C], fp8, kind="ExternalInput").ap()
    d["wp"] = nc.dram_tensor("wp", [3, 128, C], fp8, kind="ExternalInput").ap()
    d["bp"] = nc.dram_tensor("bp", [1, C], fp8, kind="ExternalInput").ap()
    d["wf1"] = nc.dram_tensor("wf1", [3, 128, HID], fp8, kind="ExternalInput").ap()
    d["bf13"] = nc.dram_tensor("bf13", [128, 12], f32, kind="ExternalInput").ap()
    d["wdg"] = nc.dram_tensor("wdg", [12, 9, 128, 128], fp8,
                              kind="ExternalInput").ap()
    d["bdw3"] = nc.dram_tensor("bdw3", [128, 12], f32, kind="ExternalInput").ap()
    d["wf2"] = nc.dram_tensor("wf2", [12, 128, C], fp8, kind="ExternalInput").ap()
    d["bf2"] = nc.dram_tensor("bf2", [1, C], fp8, kind="ExternalInput").ap()
    d["out"] = nc.dram_tensor("out", [BPC, N, C], f32, kind="ExternalOutput").ap()
    return d


@functools.lru_cache(maxsize=1)
def build_program(scale_items, num_devices=NCORES):
    scales = dict(scale_items)
    nc = bacc.Bacc("TRN2", target_bir_lowering=False, debug=False,
                   num_devices=num_devices)
    d = declare_tensors(nc, scales)
    with tile.TileContext(nc) as tc:
        emit_kernel(nc, tc, d)
    nc.compile()
    return nc


def _scale_for(w):
    m = float(np.abs(w).max())
    return 224.0 / m if m > 0 else 1.0


def prep_weights(inputs):
    """Host-side packing: transposes, LN folds, fp8 quantization + scales."""
    g1 = np.asarray(inputs["ln1_g"], np.float32)
    b1 = np.asarray(inputs["ln1_b"], np.float32)
    g2 = np.asarray(inputs["ln2_g"], np.float32)
    b2 = np.asarray(inputs["ln2_b"], np.float32)
    Wqkv = np.asarray(inputs["Wqkv"], np.float32)
    Wproj = np.asarray(inputs["Wproj"], np.float32)
    bproj = np.asarray(inputs["bproj"], np.float32)
    Wfc1 = np.asarray(inputs["Wfc1"], np.float32)[:, :, 0, 0]
    bfc1 = np.asarray(inputs["bfc1"], np.float32)
    Wdw = np.asarray(inputs["Wdw"], np.float32)[:, 0].reshape(HID, 9)
    bdw = np.asarray(inputs["bdw"], np.float32)
    Wfc2 = np.asarray(inputs["Wfc2"], np.float32)[:, :, 0, 0]
    bfc2 = np.asarray(inputs["bfc2"], np.float32)

    W3 = Wqkv.reshape(HEADS, 3, HD, C)      # out channel o = h*288 + s*96 + d
    scale = float(HD) ** -0.5
    Wq = W3[:, 0].reshape(HEADS * HD, C)
    Wk = W3[:, 1].reshape(HEADS * HD, C)
    Wv = W3[:, 2].reshape(HEADS * HD, C)

    wqk_full = np.concatenate([Wq * g1[None, :] * scale, Wk * g1[None, :]], 0)
    wv_full = Wv * g1[None, :]
    wf1_full = Wfc1 * g2[None, :]
    wdw_full = Wdw / 6.0
    wf2_full = Wfc2 / 6.0

    sc = {
        "qk": _scale_for(wqk_full), "v": _scale_for(wv_full),
        "p": _scale_for(Wproj), "f1": _scale_for(wf1_full),
        "dw": _scale_for(wdw_full), "f2": _scale_for(wf2_full),
    }

    d = {}
    d["wqk"] = np.ascontiguousarray(
        (wqk_full * sc["qk"]).T.reshape(3, 128, 2 * C)).astype(nf8)
    d["bqk"] = np.ascontiguousarray(np.concatenate(
        [((Wq @ b1) * scale).reshape(HEADS, HD).T,
         (Wk @ b1).reshape(HEADS, HD).T], 1)).astype(np.float32)
    d["wv"] = np.ascontiguousarray(
        (wv_full * sc["v"]).T.reshape(3, 128, C)).astype(nf8)
    d["bv"] = ((Wv @ b1) * sc["v"])[None, :].astype(nf8)
    d["wp"] = np.ascontiguousarray(
        (Wproj * sc["p"]).T.reshape(3, 128, C)).astype(nf8)
    d["bp"] = (bproj * sc["p"])[None, :].astype(nf8)
    d["wf1"] = np.ascontiguousarray(
        (wf1_full * sc["f1"]).T.reshape(3, 128, HID)).astype(nf8)
    d["bf13"] = np.ascontiguousarray(
        (bfc1 + Wfc1 @ b2 + 3.0).reshape(12, 128).T).astype(np.float32)
    wd = wdw_full * sc["dw"]
    wdg = np.zeros((12, 9, 128, 128), np.float32)
    ii = np.arange(128)
    for m in range(12):
        for t in range(9):
            wdg[m, t, ii, ii] = wd[m * 128 + ii, t]
    d["wdg"] = wdg.astype(nf8)
    d["bdw3"] = np.ascontiguousarray(
        (bdw + 3.0).reshape(12, 128).T).astype(np.float32)
    d["wf2"] = np.ascontiguousarray(
        (wf2_full * sc["f2"]).T.reshape(12, 128, C)).astype(nf8)
    d["bf2"] = (bfc2 * sc["f2"])[None, :].astype(nf8)
    return d, sc


def kernel(**inputs):
    from concourse.bass_utils import run_bass_kernel_spmd

    x = np.asarray(inputs["x"], np.float32)
    wd, sc = prep_weights(inputs)
    nc = build_program(tuple(sorted(sc.items())))
    in_maps = []
    for c in range(NCORES):
        m = dict(wd)
        m["x"] = np.ascontiguousarray(x[c * BPC:(c + 1) * BPC])
        in_maps.append(m)
    res = run_bass_kernel_spmd(nc, in_maps, list(range(NCORES)))
    out = np.concatenate([res.results[c]["out"] for c in range(NCORES)], axis=0)
    return out.astype(np.float32)
